# revision 10
# baseline (speedup 1.0000x reference)
"""GaussianImage (Cholesky) renderer on 8 trn2 NeuronCores.

Tile-parallel over the pixel grid: the 256x256 image is cut into 32x32
tiles (64/frame, 128 total).  The host bins gaussians to tiles (pure
routing via a conservative support radius), then greedily bin-packs
whole tiles into 128-slot "groups" so the partition dim is ~full of real
gaussians (vs. the old one-tile-per-128-slots padding).  Each core runs
G groups.  All value math runs on device:

  per gaussian slot : sigmoid / conic / quadratic-basis coeffs   [Vector/Scalar]
  per group         : sigma = (c_hi + c_lo)(6,128)^T @ basis(6,1024)
                      -- two accumulating bf16 matmuls; the centered
                         integer basis is exact in bf16, and c_hi/c_lo
                         splitting restores fp32-level coefficients,
                         avoiding the 2-pass fp32 LOW_HIGH mode
                      alpha = Exp(-sigma)              -> bf16 [ScalarE]
                      img   = wblk(128,3K)^T @ alpha(128,1024)  [TensorE]
                      -- wblk is block-diagonal per tile (built from a
                         host-routed 0/1 mask so the SPMD program is
                         identical on every core)
                      out   = clamp(img, 0, 1)                  [VectorE]

Each pixel is owned by exactly one tile -> no cross-core reduction.
"""

import os
import numpy as np
import ml_dtypes

T, N, H, W = 2, 512, 256, 256
TILE = 32
NT = H // TILE          # 8 tiles per axis
N_CORES = 8
SLOTS = 128
PIX = TILE * TILE       # 1024
SIGMA_CUT = 18.0        # exp(-18) ~ 1.5e-8: invisible at fp32 image scale

_CACHE = {}


NWARM = 8               # PE warmup matmuls: ramp the p-state to 2.4 GHz


def _build_nc(G, K):
    import concourse.bass as bass
    import concourse.mybir as mybir
    from concourse.tile import TileContext
    import bass_rust

    f32 = mybir.dt.float32
    bf16 = mybir.dt.bfloat16
    Alu = mybir.AluOpType
    Act = mybir.ActivationFunctionType

    nc = bass.Bass("TRN2")
    params = nc.dram_tensor("params", [SLOTS, G * 12], f32, kind="ExternalInput")
    basis = nc.dram_tensor("basis", [6, PIX], bf16, kind="ExternalInput")
    msk = nc.dram_tensor("msk", [SLOTS, G * 3 * K], bf16, kind="ExternalInput")
    ident = nc.dram_tensor("ident", [SLOTS, SLOTS], f32, kind="ExternalInput")
    out = nc.dram_tensor("out", [3 * K, G * PIX], f32, kind="ExternalOutput")

    with TileContext(nc) as tc:
        with tc.tile_pool(name="const", bufs=1) as cpool, \
             tc.tile_pool(name="work", bufs=3) as wpool, \
             tc.tile_pool(name="ps_sig", bufs=2, space="PSUM") as ps_sig_pool, \
             tc.tile_pool(name="ps_img", bufs=2, space="PSUM") as ps_img_pool:

            p3 = cpool.tile([SLOTS, G, 12], f32, tag="params")
            bt = cpool.tile([6, PIX], bf16, tag="basis")
            mt = cpool.tile([SLOTS, G, 3 * K], bf16, tag="msk")
            it = cpool.tile([SLOTS, SLOTS], f32, tag="ident")
            # spread input DMAs over engine queues so they run concurrently
            nc.sync.dma_start(out=p3, in_=params[:].rearrange("p (g k) -> p g k", k=12))
            nc.gpsimd.dma_start(out=bt, in_=basis[:])
            nc.gpsimd.dma_start(out=mt, in_=msk[:].rearrange("p (g k) -> p g k", k=3 * K))
            nc.sync.dma_start(out=it, in_=ident[:])

            V = nc.vector
            S = nc.scalar
            GP = nc.gpsimd

            # warm the ACT table set while the params DMA is in flight
            warm = cpool.tile([SLOTS, 1], f32, tag="warm")
            S.memzero(warm)
            S.activation(warm, warm, Act.Sigmoid)

            def sc(tag):
                return cpool.tile([SLOTS, G], f32, tag=tag, name=tag)

            cth = cpool.tile([SLOTS, G, 6], f32, tag="coef")
            wsg = cpool.tile([SLOTS, G, 3], f32, tag="wsg")
            w3 = cpool.tile([SLOTS, G, 3], f32, tag="w3")

            # centers: cx = 0.5*W*(tanh(z)+1) = W*sigmoid(2z); offsets absorbed
            sx, sy = sc("sx"), sc("sy")
            S.activation(sx, p3[:, :, 0], Act.Sigmoid, scale=2.0)
            S.activation(sy, p3[:, :, 1], Act.Sigmoid, scale=2.0)
            # sxy^2 and syy parts via ScalarE square (bias folds the +0.5)
            hf = cpool.tile([SLOTS, 1], f32, tag="hf")
            GP.memset(hf, 0.5)
            t2, t3 = sc("t2"), sc("t3")
            S.activation(t2, p3[:, :, 3], Act.Square)
            S.activation(t3, p3[:, :, 4], Act.Square, bias=hf)
            osg = sc("osg")
            S.activation(osg, p3[:, :, 5], Act.Sigmoid)
            S.activation(wsg, p3[:, :, 6:9], Act.Sigmoid)
            # pre-load the Exp ACT table while ScalarE is otherwise idle so it
            # doesn't serialize inside the hot loop
            S.activation(warm, warm, Act.Exp)

            # conic coefficient chain: det path + linear terms on Vector,
            # head + quadratic branch on GpSimd (plain tensor-tensor only)
            a0 = sc("a0")
            GP.tensor_add(out=a0, in0=p3[:, :, 2], in1=hf.broadcast_to([SLOTS, G]))
            a1 = p3[:, :, 3]
            t0, t1, v = sc("t0"), sc("t1"), sc("v")
            GP.tensor_mul(out=t0, in0=a0, in1=a0)
            GP.tensor_mul(out=t1, in0=a0, in1=a1)
            GP.tensor_mul(out=v, in0=t1, in1=t1)
            ex, ey = sc("ex"), sc("ey")
            V.scalar_tensor_tensor(out=ex, in0=sx, scalar=float(W), in1=p3[:, :, 9],
                                   op0=Alu.mult, op1=Alu.subtract)
            V.scalar_tensor_tensor(out=ey, in0=sy, scalar=float(H), in1=p3[:, :, 10],
                                   op0=Alu.mult, op1=Alu.subtract)
            syy = sc("syy")
            V.tensor_add(out=syy, in0=t2, in1=t3)
            u, det, rdet = sc("u"), sc("det"), sc("rdet")
            V.tensor_mul(out=u, in0=t0, in1=syy)
            V.tensor_sub(out=det, in0=u, in1=v)
            V.reciprocal(out=rdet, in_=det)
            # cth0 = 0.5*ca, cth1 = cb = -sxy/det, cth2 = 0.5*cc
            V.scalar_tensor_tensor(out=cth[:, :, 0], in0=syy, scalar=0.5, in1=rdet,
                                   op0=Alu.mult, op1=Alu.mult)
            V.scalar_tensor_tensor(out=cth[:, :, 1], in0=t1, scalar=-1.0, in1=rdet,
                                   op0=Alu.mult, op1=Alu.mult)
            V.scalar_tensor_tensor(out=cth[:, :, 2], in0=t0, scalar=0.5, in1=rdet,
                                   op0=Alu.mult, op1=Alu.mult)
            # linear terms on Vector via fused scalar-tensor-tensor:
            # coef_x = (-2ex)*cth0 + (-ey)*cth1, coef_y = (-2ey)*cth2 + (-ex)*cth1
            m1, m2 = sc("m1"), sc("m2")
            V.scalar_tensor_tensor(out=m1, in0=ex, scalar=-2.0, in1=cth[:, :, 0],
                                   op0=Alu.mult, op1=Alu.mult)
            V.scalar_tensor_tensor(out=m2, in0=ey, scalar=-1.0, in1=cth[:, :, 1],
                                   op0=Alu.mult, op1=Alu.mult)
            V.tensor_add(out=cth[:, :, 3], in0=m1, in1=m2)
            m3, m4 = sc("m3"), sc("m4")
            V.scalar_tensor_tensor(out=m3, in0=ey, scalar=-2.0, in1=cth[:, :, 2],
                                   op0=Alu.mult, op1=Alu.mult)
            V.scalar_tensor_tensor(out=m4, in0=ex, scalar=-1.0, in1=cth[:, :, 1],
                                   op0=Alu.mult, op1=Alu.mult)
            V.tensor_add(out=cth[:, :, 4], in0=m3, in1=m4)
            # quadratic branch on GpSimd: cth5 = cth0*ex^2 + cth1*ex*ey + cth2*ey^2
            exx, exy, eyy = sc("exx"), sc("exy"), sc("eyy")
            GP.tensor_mul(out=exx, in0=ex, in1=ex)
            GP.tensor_mul(out=exy, in0=ex, in1=ey)
            GP.tensor_mul(out=eyy, in0=ey, in1=ey)
            p1, p2, p3b, q = sc("p1"), sc("p2"), sc("p3b"), sc("q")
            GP.tensor_mul(out=p1, in0=cth[:, :, 0], in1=exx)
            GP.tensor_mul(out=p2, in0=cth[:, :, 1], in1=exy)
            GP.tensor_add(out=q, in0=p1, in1=p2)
            GP.tensor_mul(out=p3b, in0=cth[:, :, 2], in1=eyy)
            GP.tensor_add(out=cth[:, :, 5], in0=q, in1=p3b)

            # colors*opacity, scattered block-diagonally through the mask
            # (0-stride broadcasts turn 27 tiny muls into 1 + G)
            V.tensor_mul(out=w3, in0=wsg,
                         in1=osg.unsqueeze(2).broadcast_to([SLOTS, G, 3]))
            wblk = cpool.tile([SLOTS, G, 3 * K], bf16, tag="wblk")
            for g in range(G):
                GP.tensor_mul(
                    out=wblk[:, g, :].rearrange("p (k c) -> p k c", c=3),
                    in0=w3[:, g, :].unsqueeze(1).broadcast_to([SLOTS, K, 3]),
                    in1=mt[:, g, :].rearrange("p (k c) -> p k c", c=3))

            # per-group transpose of coeffs -> [6,128] at partition 0 (no
            # shuffle DMA needed; transposed tiles are valid lhsT operands)
            chis, clos = [], []
            for g in range(G):
                tpg = ps_img_pool.tile([6, SLOTS], f32, tag="img", name=f"tp{g}")
                nc.tensor.transpose(tpg, cth[:, g, :], it)
                chi = cpool.tile([6, SLOTS], bf16, tag=f"chi{g}", name=f"chi{g}")
                clo = cpool.tile([6, SLOTS], bf16, tag=f"clo{g}", name=f"clo{g}")
                S.copy(out=chi, in_=tpg)
                V.tensor_sub(out=clo, in0=tpg, in1=chi)
                chis.append(chi)
                clos.append(clo)

            st = cpool.tile([3 * K, G * PIX], f32, tag="stage")
            alphas = []

            # --- hot loop, PE-dense order: sig0 sig1 img0 sig2 img1 img2;
            # exp and clamp pipelined at 512-column half granularity ---
            def do_sigma(g):
                sig = ps_sig_pool.tile([SLOTS, PIX], f32, tag="sig", name=f"sig{g}")
                alpha = wpool.tile([SLOTS, PIX], bf16, tag="alpha", name=f"alpha{g}")
                for h in range(2):
                    cs = slice(512 * h, 512 * (h + 1))
                    nc.tensor.matmul(sig[:, cs], chis[g], bt[:, cs], start=True, stop=False)
                    nc.tensor.matmul(sig[:, cs], clos[g], bt[:, cs], start=False, stop=True)
                    S.activation(alpha[:, cs], sig[:, cs], Act.Exp, scale=-1.0)
                alphas.append(alpha)

            def do_img(g):
                img = ps_img_pool.tile([3 * K, PIX], f32, tag="img", name=f"img{g}")
                wre = wblk[:, g, :]
                o0 = g * PIX
                for h in range(2):
                    cs = slice(512 * h, 512 * (h + 1))
                    nc.tensor.matmul(img[:, cs], wre, alphas[g][:, cs], start=True, stop=True)
                    V.tensor_scalar(out=st[:, o0 + 512 * h:o0 + 512 * (h + 1)], in0=img[:, cs],
                                    scalar1=0.0, scalar2=1.0, op0=Alu.max, op1=Alu.min)
                nc.sync.dma_start(out=out[:, o0:o0 + PIX], in_=st[:, o0:o0 + PIX])

            do_sigma(0)
            do_sigma(1)
            do_img(0)
            if G > 2:
                do_sigma(2)
            do_img(1)
            if G > 2:
                do_img(2)
            for g in range(3, G):
                do_sigma(g)
                do_img(g)

    bass_rust.generate_event_semaphores(nc)
    return nc


def _bin_entries(xyz, cholesky):
    """Host-side routing: which gaussians overlap which 32x32 tile."""
    means = np.tanh(xyz.astype(np.float64))
    cx = 0.5 * W * (means[..., 0] + 1.0)
    cy = 0.5 * H * (means[..., 1] + 1.0)
    chol = cholesky.astype(np.float64) + np.array([0.5, 0.0, 0.5])
    l0, l1, l2 = chol[..., 0], chol[..., 1], chol[..., 2]
    sxx, sxy, syy = l0 * l0, l0 * l1, l1 * l1 + l2 * l2
    tr, det = sxx + syy, sxx * syy - sxy * sxy
    lam = tr / 2 + np.sqrt(np.maximum(tr * tr / 4 - det, 0.0))
    r = np.sqrt(2.0 * SIGMA_CUT * np.maximum(lam, 0.0)) + 1.0

    entries = []  # (frame, ty, tx, index-list)
    for t in range(T):
        x0 = np.clip(((cx[t] - r[t]) // TILE).astype(int), 0, NT - 1)
        x1 = np.clip(((cx[t] + r[t]) // TILE).astype(int), 0, NT - 1)
        y0 = np.clip(((cy[t] - r[t]) // TILE).astype(int), 0, NT - 1)
        y1 = np.clip(((cy[t] + r[t]) // TILE).astype(int), 0, NT - 1)
        buckets = [[[] for _ in range(NT)] for _ in range(NT)]
        for n in range(N):
            for ty in range(y0[n], y1[n] + 1):
                for tx in range(x0[n], x1[n] + 1):
                    buckets[ty][tx].append(n)
        for ty in range(NT):
            for tx in range(NT):
                assert len(buckets[ty][tx]) <= SLOTS, "tile overflow: >128 gaussians"
                if buckets[ty][tx]:
                    entries.append((t, ty, tx, buckets[ty][tx]))
    return entries


def _pack_groups(entries):
    """Greedy first-fit-decreasing bin pack of tiles into 128-slot groups."""
    order = sorted(range(len(entries)), key=lambda i: -len(entries[i][3]))
    groups = []  # [used_slots, [(entry_idx, slot_start), ...]]
    for i in order:
        c = len(entries[i][3])
        for gr in groups:
            if gr[0] + c <= SLOTS:
                gr[1].append((i, gr[0]))
                gr[0] += c
                break
        else:
            groups.append([c, [(i, 0)]])
    return groups


def _ensure_ntff_hook():
    """Provide antenv.axon_hooks (missing in this image) so trace=True works."""
    import sys, types, ctypes, contextlib
    if "antenv.axon_hooks" in sys.modules:
        return
    so_path = "/opt/axon/libaxon_pjrt.so"
    if not os.path.exists(so_path):
        return
    lib = ctypes.CDLL(so_path)
    if not hasattr(lib, "axon_start_nrt_profile"):
        return
    lib.axon_start_nrt_profile.argtypes = [ctypes.POINTER(ctypes.c_int64), ctypes.c_size_t]
    lib.axon_start_nrt_profile.restype = ctypes.c_int64
    lib.axon_stop_nrt_profile.argtypes = [ctypes.c_char_p]
    lib.axon_stop_nrt_profile.restype = ctypes.c_int64

    @contextlib.contextmanager
    def _hook(output_dir, device_ids):
        import jax
        jax.devices()
        if device_ids:
            ids = (ctypes.c_int64 * len(device_ids))(*device_ids)
            rc = lib.axon_start_nrt_profile(ids, len(device_ids))
        else:
            rc = lib.axon_start_nrt_profile(None, 0)
        if rc != 0:
            raise RuntimeError(f"axon_start_nrt_profile rc={rc}")
        try:
            yield
        finally:
            n = lib.axon_stop_nrt_profile(str(output_dir).encode())
            print(f"profile: {n} file(s) written to {output_dir}")

    mod = types.ModuleType("antenv.axon_hooks")
    mod.get_axon_ntff_profile_hook = lambda: _hook
    mod.set_axon_ntff_profile_hook = lambda h: None
    sys.modules["antenv.axon_hooks"] = mod


def kernel(xyz, cholesky, opacity, features_dc):
    from concourse import bass_utils

    xyz = np.asarray(xyz, np.float32)
    cholesky = np.asarray(cholesky, np.float32)
    opacity = np.asarray(opacity, np.float32)
    features_dc = np.asarray(features_dc, np.float32)

    entries = _bin_entries(xyz, cholesky)
    groups = _pack_groups(entries)
    G = (len(groups) + N_CORES - 1) // N_CORES
    K = max(len(gr[1]) for gr in groups)

    # centered integer basis: exact in bf16
    gx = (np.arange(PIX) % TILE - 16).astype(np.float32)
    gy = (np.arange(PIX) // TILE - 16).astype(np.float32)
    basis = np.stack([gx * gx, gx * gy, gy * gy, gx, gy,
                      np.ones(PIX, np.float32)]).astype(ml_dtypes.bfloat16)
    ident = np.eye(SLOTS, dtype=np.float32)

    in_maps = []
    unpack = []  # per core: list of (g, j, t, ty, tx)
    for c in range(N_CORES):
        pm = np.zeros((SLOTS, G, 12), np.float32)
        mk = np.zeros((SLOTS, G, 3 * K), np.float32)
        um = []
        for g in range(G):
            gi = c + g * N_CORES
            if gi >= len(groups):
                continue
            for j, (ei, s0) in enumerate(groups[gi][1]):
                t, ty, tx, idxs = entries[ei]
                ns = len(idxs)
                ids = np.asarray(idxs)
                pm[s0:s0 + ns, g, 0:2] = xyz[t, ids]
                pm[s0:s0 + ns, g, 2:5] = cholesky[t, ids]
                pm[s0:s0 + ns, g, 5] = opacity[ids, 0]
                pm[s0:s0 + ns, g, 6:9] = features_dc[ids]
                pm[s0:s0 + ns, g, 9] = tx * TILE + 16.0
                pm[s0:s0 + ns, g, 10] = ty * TILE + 16.0
                mk[s0:s0 + ns, g, 3 * j:3 * j + 3] = 1.0
                um.append((g, j, t, ty, tx))
        in_maps.append({"params": pm.reshape(SLOTS, G * 12),
                        "basis": basis,
                        "msk": mk.reshape(SLOTS, G * 3 * K).astype(ml_dtypes.bfloat16),
                        "ident": ident})
        unpack.append(um)

    if (G, K) not in _CACHE:
        _CACHE[(G, K)] = _build_nc(G, K)
    nc = _CACHE[(G, K)]

    trace = bool(int(os.environ.get("GS_TRACE", "0")))
    if trace:
        _ensure_ntff_hook()
    res = bass_utils.run_bass_kernel_spmd(
        nc, in_maps, core_ids=list(range(N_CORES)), trace=trace)
    kernel.last_result = res

    img = np.zeros((T, 3, H, W), np.float32)
    for c in range(N_CORES):
        o = res.results[c]["out"]
        for (g, j, t, ty, tx) in unpack[c]:
            img[t, :, ty * TILE:(ty + 1) * TILE, tx * TILE:(tx + 1) * TILE] = \
                o[3 * j:3 * j + 3, g * PIX:(g + 1) * PIX].reshape(3, TILE, TILE)
    return img


# revision 11
# speedup vs baseline: 1.2567x; 1.2567x over previous
"""GaussianImage (Cholesky) renderer on 8 trn2 NeuronCores.

Tile-parallel over the pixel grid: the 256x256 image is cut into 32x32
tiles (64/frame, 128 total).  The host bins gaussians to tiles (pure
routing via a conservative support radius), then greedily bin-packs
whole tiles into 128-slot "groups" so the partition dim is ~full of real
gaussians (vs. the old one-tile-per-128-slots padding).  Each core runs
G groups.  All value math runs on device:

  per gaussian slot : sigmoid / conic / quadratic-basis coeffs   [Vector/Scalar]
  per group         : sigma = (c_hi + c_lo)(6,128)^T @ basis(6,1024)
                      -- two accumulating bf16 matmuls; the centered
                         integer basis is exact in bf16, and c_hi/c_lo
                         splitting restores fp32-level coefficients,
                         avoiding the 2-pass fp32 LOW_HIGH mode
                      alpha = Exp(-sigma)              -> bf16 [ScalarE]
                      img   = wblk(128,3K)^T @ alpha(128,1024)  [TensorE]
                      -- wblk is block-diagonal per tile (built from a
                         host-routed 0/1 mask so the SPMD program is
                         identical on every core)
                      out   = clamp(img, 0, 1)                  [VectorE]

Each pixel is owned by exactly one tile -> no cross-core reduction.
"""

import os
import numpy as np
import ml_dtypes

T, N, H, W = 2, 512, 256, 256
TILE = 32
NT = H // TILE          # 8 tiles per axis
N_CORES = 8
SLOTS = 128
PIX = TILE * TILE       # 1024
SIGMA_CUT = 18.0        # exp(-18) ~ 1.5e-8: invisible at fp32 image scale

_CACHE = {}


NWARM = 8               # PE warmup matmuls: ramp the p-state to 2.4 GHz


def _build_nc(G, K):
    import concourse.bass as bass
    import concourse.mybir as mybir
    from concourse.tile import TileContext
    import bass_rust

    f32 = mybir.dt.float32
    bf16 = mybir.dt.bfloat16
    Alu = mybir.AluOpType
    Act = mybir.ActivationFunctionType

    nc = bass.Bass("TRN2")
    params = nc.dram_tensor("params", [SLOTS, G * 12], f32, kind="ExternalInput")
    basis = nc.dram_tensor("basis", [6, PIX], bf16, kind="ExternalInput")
    msk = nc.dram_tensor("msk", [SLOTS, G * 3 * K], bf16, kind="ExternalInput")
    ident = nc.dram_tensor("ident", [SLOTS, SLOTS], f32, kind="ExternalInput")
    out = nc.dram_tensor("out", [3 * K, G * PIX], f32, kind="ExternalOutput")

    with TileContext(nc) as tc:
        with tc.tile_pool(name="const", bufs=1) as cpool, \
             tc.tile_pool(name="work", bufs=3) as wpool, \
             tc.tile_pool(name="ps_sig", bufs=2, space="PSUM") as ps_sig_pool, \
             tc.tile_pool(name="ps_img", bufs=2, space="PSUM") as ps_img_pool:

            p3 = cpool.tile([SLOTS, G, 12], f32, tag="params")
            bt = cpool.tile([6, PIX], bf16, tag="basis")
            mt = cpool.tile([SLOTS, G, 3 * K], bf16, tag="msk")
            it = cpool.tile([SLOTS, SLOTS], f32, tag="ident")
            # spread input DMAs over engine queues so they run concurrently
            nc.sync.dma_start(out=p3, in_=params[:].rearrange("p (g k) -> p g k", k=12))
            nc.gpsimd.dma_start(out=bt, in_=basis[:])
            nc.gpsimd.dma_start(out=mt, in_=msk[:].rearrange("p (g k) -> p g k", k=3 * K))
            nc.sync.dma_start(out=it, in_=ident[:])

            V = nc.vector
            S = nc.scalar
            GP = nc.gpsimd

            # warm the ACT table set while the params DMA is in flight
            warm = cpool.tile([SLOTS, 1], f32, tag="warm")
            S.memzero(warm)
            S.activation(warm, warm, Act.Sigmoid)

            def sc(tag):
                return cpool.tile([SLOTS, G], f32, tag=tag, name=tag)

            cth = cpool.tile([SLOTS, G, 6], f32, tag="coef")
            wsg = cpool.tile([SLOTS, G, 3], f32, tag="wsg")
            w3 = cpool.tile([SLOTS, G, 3], f32, tag="w3")

            # centers: cx = 0.5*W*(tanh(z)+1) = W*sigmoid(2z); offsets absorbed
            sx, sy = sc("sx"), sc("sy")
            S.activation(sx, p3[:, :, 0], Act.Sigmoid, scale=2.0)
            S.activation(sy, p3[:, :, 1], Act.Sigmoid, scale=2.0)
            # sxy^2 and syy parts via ScalarE square (bias folds the +0.5)
            hf = cpool.tile([SLOTS, 1], f32, tag="hf")
            GP.memset(hf, 0.5)
            t2, t3 = sc("t2"), sc("t3")
            S.activation(t2, p3[:, :, 3], Act.Square)
            S.activation(t3, p3[:, :, 4], Act.Square, bias=hf)
            osg = sc("osg")
            S.activation(osg, p3[:, :, 5], Act.Sigmoid)
            S.activation(wsg, p3[:, :, 6:9], Act.Sigmoid)
            # pre-load the Exp ACT table while ScalarE is otherwise idle so it
            # doesn't serialize inside the hot loop.  Reading wsg (the last
            # sigmoid output) pins this AFTER all sigmoid-set activations —
            # ScalarE holds one table set, every set switch costs ~1.3us.
            S.activation(warm, wsg[:, 0, 0:1], Act.Exp)

            # conic coefficient chain: det path + linear terms on Vector,
            # head + quadratic branch on GpSimd (plain tensor-tensor only)
            a0 = sc("a0")
            GP.tensor_add(out=a0, in0=p3[:, :, 2], in1=hf.broadcast_to([SLOTS, G]))
            a1 = p3[:, :, 3]
            t0, t1, v = sc("t0"), sc("t1"), sc("v")
            GP.tensor_mul(out=t0, in0=a0, in1=a0)
            GP.tensor_mul(out=t1, in0=a0, in1=a1)
            GP.tensor_mul(out=v, in0=t1, in1=t1)
            ex, ey = sc("ex"), sc("ey")
            V.scalar_tensor_tensor(out=ex, in0=sx, scalar=float(W), in1=p3[:, :, 9],
                                   op0=Alu.mult, op1=Alu.subtract)
            V.scalar_tensor_tensor(out=ey, in0=sy, scalar=float(H), in1=p3[:, :, 10],
                                   op0=Alu.mult, op1=Alu.subtract)
            syy = sc("syy")
            V.tensor_add(out=syy, in0=t2, in1=t3)
            u, det, rdet = sc("u"), sc("det"), sc("rdet")
            V.tensor_mul(out=u, in0=t0, in1=syy)
            V.tensor_sub(out=det, in0=u, in1=v)
            V.reciprocal(out=rdet, in_=det)
            # cth0 = 0.5*ca, cth1 = cb = -sxy/det, cth2 = 0.5*cc
            V.scalar_tensor_tensor(out=cth[:, :, 0], in0=syy, scalar=0.5, in1=rdet,
                                   op0=Alu.mult, op1=Alu.mult)
            V.scalar_tensor_tensor(out=cth[:, :, 1], in0=t1, scalar=-1.0, in1=rdet,
                                   op0=Alu.mult, op1=Alu.mult)
            V.scalar_tensor_tensor(out=cth[:, :, 2], in0=t0, scalar=0.5, in1=rdet,
                                   op0=Alu.mult, op1=Alu.mult)
            # linear terms on Vector via fused scalar-tensor-tensor:
            # coef_x = (-2ex)*cth0 + (-ey)*cth1, coef_y = (-2ey)*cth2 + (-ex)*cth1
            m1, m2 = sc("m1"), sc("m2")
            V.scalar_tensor_tensor(out=m1, in0=ex, scalar=-2.0, in1=cth[:, :, 0],
                                   op0=Alu.mult, op1=Alu.mult)
            V.scalar_tensor_tensor(out=m2, in0=ey, scalar=-1.0, in1=cth[:, :, 1],
                                   op0=Alu.mult, op1=Alu.mult)
            V.tensor_add(out=cth[:, :, 3], in0=m1, in1=m2)
            m3, m4 = sc("m3"), sc("m4")
            V.scalar_tensor_tensor(out=m3, in0=ey, scalar=-2.0, in1=cth[:, :, 2],
                                   op0=Alu.mult, op1=Alu.mult)
            V.scalar_tensor_tensor(out=m4, in0=ex, scalar=-1.0, in1=cth[:, :, 1],
                                   op0=Alu.mult, op1=Alu.mult)
            V.tensor_add(out=cth[:, :, 4], in0=m3, in1=m4)
            # quadratic branch on GpSimd: cth5 = cth0*ex^2 + cth1*ex*ey + cth2*ey^2
            exx, exy, eyy = sc("exx"), sc("exy"), sc("eyy")
            GP.tensor_mul(out=exx, in0=ex, in1=ex)
            GP.tensor_mul(out=exy, in0=ex, in1=ey)
            GP.tensor_mul(out=eyy, in0=ey, in1=ey)
            p1, p2, p3b, q = sc("p1"), sc("p2"), sc("p3b"), sc("q")
            GP.tensor_mul(out=p1, in0=cth[:, :, 0], in1=exx)
            GP.tensor_mul(out=p2, in0=cth[:, :, 1], in1=exy)
            GP.tensor_add(out=q, in0=p1, in1=p2)
            GP.tensor_mul(out=p3b, in0=cth[:, :, 2], in1=eyy)
            GP.tensor_add(out=cth[:, :, 5], in0=q, in1=p3b)

            # colors*opacity, scattered block-diagonally through the mask
            # (0-stride broadcasts turn 27 tiny muls into 1 + G)
            V.tensor_mul(out=w3, in0=wsg,
                         in1=osg.unsqueeze(2).broadcast_to([SLOTS, G, 3]))
            wblk = cpool.tile([SLOTS, G, 3 * K], bf16, tag="wblk")
            for g in range(G):
                GP.tensor_mul(
                    out=wblk[:, g, :].rearrange("p (k c) -> p k c", c=3),
                    in0=w3[:, g, :].unsqueeze(1).broadcast_to([SLOTS, K, 3]),
                    in1=mt[:, g, :].rearrange("p (k c) -> p k c", c=3))

            # per-group transpose of coeffs -> [6,128] at partition 0 (no
            # shuffle DMA needed; transposed tiles are valid lhsT operands)
            chis, clos = [], []
            for g in range(G):
                tpg = ps_img_pool.tile([6, SLOTS], f32, tag="img", name=f"tp{g}")
                nc.tensor.transpose(tpg, cth[:, g, :], it)
                chi = cpool.tile([6, SLOTS], bf16, tag=f"chi{g}", name=f"chi{g}")
                clo = cpool.tile([6, SLOTS], bf16, tag=f"clo{g}", name=f"clo{g}")
                S.copy(out=chi, in_=tpg)
                V.tensor_sub(out=clo, in0=tpg, in1=chi)
                chis.append(chi)
                clos.append(clo)

            st = cpool.tile([3 * K, G * PIX], f32, tag="stage")
            alphas = []

            # --- hot loop, PE-dense order: sig0 sig1 img0 sig2 img1 img2;
            # exp and clamp pipelined at 512-column half granularity ---
            def do_sigma(g):
                sig = ps_sig_pool.tile([SLOTS, PIX], f32, tag="sig", name=f"sig{g}")
                alpha = wpool.tile([SLOTS, PIX], bf16, tag="alpha", name=f"alpha{g}")
                for h in range(2):
                    cs = slice(512 * h, 512 * (h + 1))
                    nc.tensor.matmul(sig[:, cs], chis[g], bt[:, cs], start=True, stop=False)
                    nc.tensor.matmul(sig[:, cs], clos[g], bt[:, cs], start=False, stop=True)
                    S.activation(alpha[:, cs], sig[:, cs], Act.Exp, scale=-1.0)
                alphas.append(alpha)

            def do_img(g):
                img = ps_img_pool.tile([3 * K, PIX], f32, tag="img", name=f"img{g}")
                wre = wblk[:, g, :]
                o0 = g * PIX
                for h in range(2):
                    cs = slice(512 * h, 512 * (h + 1))
                    nc.tensor.matmul(img[:, cs], wre, alphas[g][:, cs], start=True, stop=True)
                    V.tensor_scalar(out=st[:, o0 + 512 * h:o0 + 512 * (h + 1)], in0=img[:, cs],
                                    scalar1=0.0, scalar2=1.0, op0=Alu.max, op1=Alu.min)
                nc.sync.dma_start(out=out[:, o0:o0 + PIX], in_=st[:, o0:o0 + PIX])

            do_sigma(0)
            do_sigma(1)
            do_img(0)
            if G > 2:
                do_sigma(2)
            do_img(1)
            if G > 2:
                do_img(2)
            for g in range(3, G):
                do_sigma(g)
                do_img(g)

    bass_rust.generate_event_semaphores(nc)
    return nc


def _bin_entries(xyz, cholesky):
    """Host-side routing: which gaussians overlap which 32x32 tile."""
    means = np.tanh(xyz.astype(np.float64))
    cx = 0.5 * W * (means[..., 0] + 1.0)
    cy = 0.5 * H * (means[..., 1] + 1.0)
    chol = cholesky.astype(np.float64) + np.array([0.5, 0.0, 0.5])
    l0, l1, l2 = chol[..., 0], chol[..., 1], chol[..., 2]
    sxx, sxy, syy = l0 * l0, l0 * l1, l1 * l1 + l2 * l2
    tr, det = sxx + syy, sxx * syy - sxy * sxy
    lam = tr / 2 + np.sqrt(np.maximum(tr * tr / 4 - det, 0.0))
    r = np.sqrt(2.0 * SIGMA_CUT * np.maximum(lam, 0.0)) + 1.0

    entries = []  # (frame, ty, tx, index-list)
    for t in range(T):
        x0 = np.clip(((cx[t] - r[t]) // TILE).astype(int), 0, NT - 1)
        x1 = np.clip(((cx[t] + r[t]) // TILE).astype(int), 0, NT - 1)
        y0 = np.clip(((cy[t] - r[t]) // TILE).astype(int), 0, NT - 1)
        y1 = np.clip(((cy[t] + r[t]) // TILE).astype(int), 0, NT - 1)
        buckets = [[[] for _ in range(NT)] for _ in range(NT)]
        for n in range(N):
            for ty in range(y0[n], y1[n] + 1):
                for tx in range(x0[n], x1[n] + 1):
                    buckets[ty][tx].append(n)
        for ty in range(NT):
            for tx in range(NT):
                assert len(buckets[ty][tx]) <= SLOTS, "tile overflow: >128 gaussians"
                if buckets[ty][tx]:
                    entries.append((t, ty, tx, buckets[ty][tx]))
    return entries


def _pack_groups(entries):
    """Greedy first-fit-decreasing bin pack of tiles into 128-slot groups."""
    order = sorted(range(len(entries)), key=lambda i: -len(entries[i][3]))
    groups = []  # [used_slots, [(entry_idx, slot_start), ...]]
    for i in order:
        c = len(entries[i][3])
        for gr in groups:
            if gr[0] + c <= SLOTS:
                gr[1].append((i, gr[0]))
                gr[0] += c
                break
        else:
            groups.append([c, [(i, 0)]])
    return groups


def _ensure_ntff_hook():
    """Provide antenv.axon_hooks (missing in this image) so trace=True works."""
    import sys, types, ctypes, contextlib
    if "antenv.axon_hooks" in sys.modules:
        return
    so_path = "/opt/axon/libaxon_pjrt.so"
    if not os.path.exists(so_path):
        return
    lib = ctypes.CDLL(so_path)
    if not hasattr(lib, "axon_start_nrt_profile"):
        return
    lib.axon_start_nrt_profile.argtypes = [ctypes.POINTER(ctypes.c_int64), ctypes.c_size_t]
    lib.axon_start_nrt_profile.restype = ctypes.c_int64
    lib.axon_stop_nrt_profile.argtypes = [ctypes.c_char_p]
    lib.axon_stop_nrt_profile.restype = ctypes.c_int64

    @contextlib.contextmanager
    def _hook(output_dir, device_ids):
        import jax
        jax.devices()
        if device_ids:
            ids = (ctypes.c_int64 * len(device_ids))(*device_ids)
            rc = lib.axon_start_nrt_profile(ids, len(device_ids))
        else:
            rc = lib.axon_start_nrt_profile(None, 0)
        if rc != 0:
            raise RuntimeError(f"axon_start_nrt_profile rc={rc}")
        try:
            yield
        finally:
            n = lib.axon_stop_nrt_profile(str(output_dir).encode())
            print(f"profile: {n} file(s) written to {output_dir}")

    mod = types.ModuleType("antenv.axon_hooks")
    mod.get_axon_ntff_profile_hook = lambda: _hook
    mod.set_axon_ntff_profile_hook = lambda h: None
    sys.modules["antenv.axon_hooks"] = mod


def kernel(xyz, cholesky, opacity, features_dc):
    from concourse import bass_utils

    xyz = np.asarray(xyz, np.float32)
    cholesky = np.asarray(cholesky, np.float32)
    opacity = np.asarray(opacity, np.float32)
    features_dc = np.asarray(features_dc, np.float32)

    entries = _bin_entries(xyz, cholesky)
    groups = _pack_groups(entries)
    G = (len(groups) + N_CORES - 1) // N_CORES
    K = max(len(gr[1]) for gr in groups)

    # centered integer basis: exact in bf16
    gx = (np.arange(PIX) % TILE - 16).astype(np.float32)
    gy = (np.arange(PIX) // TILE - 16).astype(np.float32)
    basis = np.stack([gx * gx, gx * gy, gy * gy, gx, gy,
                      np.ones(PIX, np.float32)]).astype(ml_dtypes.bfloat16)
    ident = np.eye(SLOTS, dtype=np.float32)

    in_maps = []
    unpack = []  # per core: list of (g, j, t, ty, tx)
    for c in range(N_CORES):
        pm = np.zeros((SLOTS, G, 12), np.float32)
        mk = np.zeros((SLOTS, G, 3 * K), np.float32)
        um = []
        for g in range(G):
            gi = c + g * N_CORES
            if gi >= len(groups):
                continue
            for j, (ei, s0) in enumerate(groups[gi][1]):
                t, ty, tx, idxs = entries[ei]
                ns = len(idxs)
                ids = np.asarray(idxs)
                pm[s0:s0 + ns, g, 0:2] = xyz[t, ids]
                pm[s0:s0 + ns, g, 2:5] = cholesky[t, ids]
                pm[s0:s0 + ns, g, 5] = opacity[ids, 0]
                pm[s0:s0 + ns, g, 6:9] = features_dc[ids]
                pm[s0:s0 + ns, g, 9] = tx * TILE + 16.0
                pm[s0:s0 + ns, g, 10] = ty * TILE + 16.0
                mk[s0:s0 + ns, g, 3 * j:3 * j + 3] = 1.0
                um.append((g, j, t, ty, tx))
        in_maps.append({"params": pm.reshape(SLOTS, G * 12),
                        "basis": basis,
                        "msk": mk.reshape(SLOTS, G * 3 * K).astype(ml_dtypes.bfloat16),
                        "ident": ident})
        unpack.append(um)

    if (G, K) not in _CACHE:
        _CACHE[(G, K)] = _build_nc(G, K)
    nc = _CACHE[(G, K)]

    trace = bool(int(os.environ.get("GS_TRACE", "0")))
    if trace:
        _ensure_ntff_hook()
    res = bass_utils.run_bass_kernel_spmd(
        nc, in_maps, core_ids=list(range(N_CORES)), trace=trace)
    kernel.last_result = res

    img = np.zeros((T, 3, H, W), np.float32)
    for c in range(N_CORES):
        o = res.results[c]["out"]
        for (g, j, t, ty, tx) in unpack[c]:
            img[t, :, ty * TILE:(ty + 1) * TILE, tx * TILE:(tx + 1) * TILE] = \
                o[3 * j:3 * j + 3, g * PIX:(g + 1) * PIX].reshape(3, TILE, TILE)
    return img


# revision 13
# speedup vs baseline: 1.3397x; 1.0660x over previous
"""GaussianImage (Cholesky) renderer on 8 trn2 NeuronCores.

Tile-parallel over the pixel grid: the 256x256 image is cut into 32x32
tiles (64/frame, 128 total).  The host bins gaussians to tiles (pure
routing via a conservative support radius), then greedily bin-packs
whole tiles into 128-slot "groups" so the partition dim is ~full of real
gaussians (vs. the old one-tile-per-128-slots padding).  Each core runs
G groups.  All value math runs on device:

  per gaussian slot : sigmoid / conic / quadratic-basis coeffs   [Vector/Scalar]
  per group         : sigma = (c_hi + c_lo)(6,128)^T @ basis(6,1024)
                      -- two accumulating bf16 matmuls; the centered
                         integer basis is exact in bf16, and c_hi/c_lo
                         splitting restores fp32-level coefficients,
                         avoiding the 2-pass fp32 LOW_HIGH mode
                      alpha = Exp(-sigma)              -> bf16 [ScalarE]
                      img   = wblk(128,3K)^T @ alpha(128,1024)  [TensorE]
                      -- wblk is block-diagonal per tile (built from a
                         host-routed 0/1 mask so the SPMD program is
                         identical on every core)
                      out   = clamp(img, 0, 1)                  [VectorE]

Each pixel is owned by exactly one tile -> no cross-core reduction.
"""

import os
import numpy as np
import ml_dtypes

T, N, H, W = 2, 512, 256, 256
TILE = 32
NT = H // TILE          # 8 tiles per axis
N_CORES = 8
SLOTS = 128
PIX = TILE * TILE       # 1024
SIGMA_CUT = 18.0        # exp(-18) ~ 1.5e-8: invisible at fp32 image scale

_CACHE = {}


NWARM = 8               # PE warmup matmuls: ramp the p-state to 2.4 GHz


def _build_nc(G, K):
    import concourse.bass as bass
    import concourse.mybir as mybir
    from concourse.tile import TileContext
    import bass_rust

    f32 = mybir.dt.float32
    bf16 = mybir.dt.bfloat16
    Alu = mybir.AluOpType
    Act = mybir.ActivationFunctionType

    nc = bass.Bass("TRN2")
    params = nc.dram_tensor("params", [SLOTS, G * 12], f32, kind="ExternalInput")
    basis = nc.dram_tensor("basis", [6, PIX], bf16, kind="ExternalInput")
    msk = nc.dram_tensor("msk", [SLOTS, G * 3 * K], bf16, kind="ExternalInput")
    ident = nc.dram_tensor("ident", [SLOTS, SLOTS], f32, kind="ExternalInput")
    out = nc.dram_tensor("out", [3 * K, G * PIX], f32, kind="ExternalOutput")

    with TileContext(nc) as tc:
        with tc.tile_pool(name="const", bufs=1) as cpool, \
             tc.tile_pool(name="work", bufs=3) as wpool, \
             tc.tile_pool(name="ps_sig", bufs=2, space="PSUM") as ps_sig_pool, \
             tc.tile_pool(name="ps_img", bufs=2, space="PSUM") as ps_img_pool:

            p3 = cpool.tile([SLOTS, G, 12], f32, tag="params")
            bt = cpool.tile([6, PIX], bf16, tag="basis")
            mt = cpool.tile([SLOTS, G, 3 * K], bf16, tag="msk")
            it = cpool.tile([SLOTS, SLOTS], f32, tag="ident")
            # spread input DMAs over engine queues so they run concurrently
            nc.sync.dma_start(out=p3, in_=params[:].rearrange("p (g k) -> p g k", k=12))
            nc.gpsimd.dma_start(out=bt, in_=basis[:])
            nc.gpsimd.dma_start(out=mt, in_=msk[:].rearrange("p (g k) -> p g k", k=3 * K))
            nc.sync.dma_start(out=it, in_=ident[:])

            V = nc.vector
            S = nc.scalar
            GP = nc.gpsimd

            # warm the ACT table set while the params DMA is in flight
            warm = cpool.tile([SLOTS, 1], f32, tag="warm")
            S.memzero(warm)
            S.activation(warm, warm, Act.Sigmoid)

            def sc(tag):
                return cpool.tile([SLOTS, G], f32, tag=tag, name=tag)

            cth = cpool.tile([SLOTS, G, 6], f32, tag="coef")
            wsg = cpool.tile([SLOTS, G, 3], f32, tag="wsg")
            w3 = cpool.tile([SLOTS, G, 3], f32, tag="w3")

            # centers: cx = 0.5*W*(tanh(z)+1) = W*sigmoid(2z); offsets absorbed
            sx, sy = sc("sx"), sc("sy")
            S.activation(sx, p3[:, :, 0], Act.Sigmoid, scale=2.0)
            S.activation(sy, p3[:, :, 1], Act.Sigmoid, scale=2.0)
            # sxy^2 and syy parts via ScalarE square (bias folds the +0.5)
            hf = cpool.tile([SLOTS, 1], f32, tag="hf")
            GP.memset(hf, 0.5)
            t2, t3 = sc("t2"), sc("t3")
            S.activation(t2, p3[:, :, 3], Act.Square)
            S.activation(t3, p3[:, :, 4], Act.Square, bias=hf)
            osg = sc("osg")
            S.activation(osg, p3[:, :, 5], Act.Sigmoid)
            S.activation(wsg, p3[:, :, 6:9], Act.Sigmoid)
            # pre-load the Exp ACT table while ScalarE is otherwise idle so it
            # doesn't serialize inside the hot loop.  Reading wsg (the last
            # sigmoid output) pins this AFTER all sigmoid-set activations —
            # ScalarE holds one table set, every set switch costs ~1.3us.
            S.activation(warm, wsg[:, 0, 0:1], Act.Exp)

            # conic coefficient chain: det path + linear terms on Vector,
            # head + quadratic branch on GpSimd (plain tensor-tensor only)
            a0 = sc("a0")
            GP.tensor_add(out=a0, in0=p3[:, :, 2], in1=hf.broadcast_to([SLOTS, G]))
            a1 = p3[:, :, 3]
            t0, t1, v = sc("t0"), sc("t1"), sc("v")
            GP.tensor_mul(out=t0, in0=a0, in1=a0)
            GP.tensor_mul(out=t1, in0=a0, in1=a1)
            GP.tensor_mul(out=v, in0=t1, in1=t1)
            ex, ey = sc("ex"), sc("ey")
            V.scalar_tensor_tensor(out=ex, in0=sx, scalar=float(W), in1=p3[:, :, 9],
                                   op0=Alu.mult, op1=Alu.subtract)
            V.scalar_tensor_tensor(out=ey, in0=sy, scalar=float(H), in1=p3[:, :, 10],
                                   op0=Alu.mult, op1=Alu.subtract)
            syy = sc("syy")
            V.tensor_add(out=syy, in0=t2, in1=t3)
            u, det, rdet = sc("u"), sc("det"), sc("rdet")
            V.tensor_mul(out=u, in0=t0, in1=syy)
            V.tensor_sub(out=det, in0=u, in1=v)
            V.reciprocal(out=rdet, in_=det)
            # cth0 = 0.5*ca, cth1 = cb = -sxy/det, cth2 = 0.5*cc
            V.scalar_tensor_tensor(out=cth[:, :, 0], in0=syy, scalar=0.5, in1=rdet,
                                   op0=Alu.mult, op1=Alu.mult)
            V.scalar_tensor_tensor(out=cth[:, :, 1], in0=t1, scalar=-1.0, in1=rdet,
                                   op0=Alu.mult, op1=Alu.mult)
            V.scalar_tensor_tensor(out=cth[:, :, 2], in0=t0, scalar=0.5, in1=rdet,
                                   op0=Alu.mult, op1=Alu.mult)
            # linear terms on Vector via fused scalar-tensor-tensor:
            # coef_x = (-2ex)*cth0 + (-ey)*cth1, coef_y = (-2ey)*cth2 + (-ex)*cth1
            m1, m2 = sc("m1"), sc("m2")
            V.scalar_tensor_tensor(out=m1, in0=ex, scalar=-2.0, in1=cth[:, :, 0],
                                   op0=Alu.mult, op1=Alu.mult)
            V.scalar_tensor_tensor(out=m2, in0=ey, scalar=-1.0, in1=cth[:, :, 1],
                                   op0=Alu.mult, op1=Alu.mult)
            V.tensor_add(out=cth[:, :, 3], in0=m1, in1=m2)
            m3, m4 = sc("m3"), sc("m4")
            V.scalar_tensor_tensor(out=m3, in0=ey, scalar=-2.0, in1=cth[:, :, 2],
                                   op0=Alu.mult, op1=Alu.mult)
            V.scalar_tensor_tensor(out=m4, in0=ex, scalar=-1.0, in1=cth[:, :, 1],
                                   op0=Alu.mult, op1=Alu.mult)
            V.tensor_add(out=cth[:, :, 4], in0=m3, in1=m4)
            # quadratic branch on GpSimd: cth5 = cth0*ex^2 + cth1*ex*ey + cth2*ey^2
            exx, exy, eyy = sc("exx"), sc("exy"), sc("eyy")
            GP.tensor_mul(out=exx, in0=ex, in1=ex)
            GP.tensor_mul(out=exy, in0=ex, in1=ey)
            GP.tensor_mul(out=eyy, in0=ey, in1=ey)
            p1, p2, p3b, q = sc("p1"), sc("p2"), sc("p3b"), sc("q")
            GP.tensor_mul(out=p1, in0=cth[:, :, 0], in1=exx)
            GP.tensor_mul(out=p2, in0=cth[:, :, 1], in1=exy)
            GP.tensor_add(out=q, in0=p1, in1=p2)
            GP.tensor_mul(out=p3b, in0=cth[:, :, 2], in1=eyy)
            GP.tensor_add(out=cth[:, :, 5], in0=q, in1=p3b)

            # colors*opacity, scattered block-diagonally through the mask
            # (0-stride broadcasts turn 27 tiny muls into 1 + G)
            V.tensor_mul(out=w3, in0=wsg,
                         in1=osg.unsqueeze(2).broadcast_to([SLOTS, G, 3]))
            wblk = cpool.tile([SLOTS, G, 3 * K], bf16, tag="wblk")
            for g in range(G):
                GP.tensor_mul(
                    out=wblk[:, g, :].rearrange("p (k c) -> p k c", c=3),
                    in0=w3[:, g, :].unsqueeze(1).broadcast_to([SLOTS, K, 3]),
                    in1=mt[:, g, :].rearrange("p (k c) -> p k c", c=3))

            # per-group transpose of coeffs -> [6,128] at partition 0 (no
            # shuffle DMA needed; transposed tiles are valid lhsT operands)
            chis, clos = [], []
            for g in range(G):
                tpg = ps_sig_pool.tile([6, SLOTS], f32, tag="sig", name=f"tp{g}")
                nc.tensor.transpose(tpg, cth[:, g, :], it)
                chi = cpool.tile([6, SLOTS], bf16, tag=f"chi{g}", name=f"chi{g}")
                clo = cpool.tile([6, SLOTS], bf16, tag=f"clo{g}", name=f"clo{g}")
                S.copy(out=chi, in_=tpg)
                V.tensor_sub(out=clo, in0=tpg, in1=chi)
                chis.append(chi)
                clos.append(clo)

            st = cpool.tile([3 * K, G * PIX], f32, tag="stage")
            alphas = []

            # --- hot loop, PE-dense order: sig0 sig1 img0 sig2 img1 img2;
            # exp and clamp pipelined at 512-column half granularity ---
            def do_sigma(g):
                sig = ps_sig_pool.tile([SLOTS, PIX], f32, tag="sig", name=f"sig{g}")
                alpha = wpool.tile([SLOTS, PIX], bf16, tag="alpha", name=f"alpha{g}")
                for h in range(2):
                    cs = slice(512 * h, 512 * (h + 1))
                    nc.tensor.matmul(sig[:, cs], chis[g], bt[:, cs], start=True, stop=False)
                    nc.tensor.matmul(sig[:, cs], clos[g], bt[:, cs], start=False, stop=True)
                    S.activation(alpha[:, cs], sig[:, cs], Act.Exp, scale=-1.0)
                alphas.append(alpha)

            def do_img(g):
                img = ps_img_pool.tile([3 * K, PIX], f32, tag="img", name=f"img{g}")
                wre = wblk[:, g, :]
                o0 = g * PIX
                for h in range(2):
                    cs = slice(512 * h, 512 * (h + 1))
                    nc.tensor.matmul(img[:, cs], wre, alphas[g][:, cs], start=True, stop=True)
                    V.tensor_scalar(out=st[:, o0 + 512 * h:o0 + 512 * (h + 1)], in0=img[:, cs],
                                    scalar1=0.0, scalar2=1.0, op0=Alu.max, op1=Alu.min)
                    nc.sync.dma_start(out=out[:, o0 + 512 * h:o0 + 512 * (h + 1)],
                                      in_=st[:, o0 + 512 * h:o0 + 512 * (h + 1)])

            # sigmas first, then imgs: the tail group's exp/img/clamp overlap
            # the earlier groups' img matmuls instead of serializing at the end
            for g in range(G):
                do_sigma(g)
            for g in range(G):
                do_img(g)

    bass_rust.generate_event_semaphores(nc)
    return nc


def _bin_entries(xyz, cholesky):
    """Host-side routing: which gaussians overlap which 32x32 tile."""
    means = np.tanh(xyz.astype(np.float64))
    cx = 0.5 * W * (means[..., 0] + 1.0)
    cy = 0.5 * H * (means[..., 1] + 1.0)
    chol = cholesky.astype(np.float64) + np.array([0.5, 0.0, 0.5])
    l0, l1, l2 = chol[..., 0], chol[..., 1], chol[..., 2]
    sxx, sxy, syy = l0 * l0, l0 * l1, l1 * l1 + l2 * l2
    tr, det = sxx + syy, sxx * syy - sxy * sxy
    lam = tr / 2 + np.sqrt(np.maximum(tr * tr / 4 - det, 0.0))
    r = np.sqrt(2.0 * SIGMA_CUT * np.maximum(lam, 0.0)) + 1.0

    entries = []  # (frame, ty, tx, index-list)
    for t in range(T):
        x0 = np.clip(((cx[t] - r[t]) // TILE).astype(int), 0, NT - 1)
        x1 = np.clip(((cx[t] + r[t]) // TILE).astype(int), 0, NT - 1)
        y0 = np.clip(((cy[t] - r[t]) // TILE).astype(int), 0, NT - 1)
        y1 = np.clip(((cy[t] + r[t]) // TILE).astype(int), 0, NT - 1)
        buckets = [[[] for _ in range(NT)] for _ in range(NT)]
        for n in range(N):
            for ty in range(y0[n], y1[n] + 1):
                for tx in range(x0[n], x1[n] + 1):
                    buckets[ty][tx].append(n)
        for ty in range(NT):
            for tx in range(NT):
                assert len(buckets[ty][tx]) <= SLOTS, "tile overflow: >128 gaussians"
                if buckets[ty][tx]:
                    entries.append((t, ty, tx, buckets[ty][tx]))
    return entries


def _pack_groups(entries):
    """Greedy first-fit-decreasing bin pack of tiles into 128-slot groups."""
    order = sorted(range(len(entries)), key=lambda i: -len(entries[i][3]))
    groups = []  # [used_slots, [(entry_idx, slot_start), ...]]
    for i in order:
        c = len(entries[i][3])
        for gr in groups:
            if gr[0] + c <= SLOTS:
                gr[1].append((i, gr[0]))
                gr[0] += c
                break
        else:
            groups.append([c, [(i, 0)]])
    return groups


def _ensure_ntff_hook():
    """Provide antenv.axon_hooks (missing in this image) so trace=True works."""
    import sys, types, ctypes, contextlib
    if "antenv.axon_hooks" in sys.modules:
        return
    so_path = "/opt/axon/libaxon_pjrt.so"
    if not os.path.exists(so_path):
        return
    lib = ctypes.CDLL(so_path)
    if not hasattr(lib, "axon_start_nrt_profile"):
        return
    lib.axon_start_nrt_profile.argtypes = [ctypes.POINTER(ctypes.c_int64), ctypes.c_size_t]
    lib.axon_start_nrt_profile.restype = ctypes.c_int64
    lib.axon_stop_nrt_profile.argtypes = [ctypes.c_char_p]
    lib.axon_stop_nrt_profile.restype = ctypes.c_int64

    @contextlib.contextmanager
    def _hook(output_dir, device_ids):
        import jax
        jax.devices()
        if device_ids:
            ids = (ctypes.c_int64 * len(device_ids))(*device_ids)
            rc = lib.axon_start_nrt_profile(ids, len(device_ids))
        else:
            rc = lib.axon_start_nrt_profile(None, 0)
        if rc != 0:
            raise RuntimeError(f"axon_start_nrt_profile rc={rc}")
        try:
            yield
        finally:
            n = lib.axon_stop_nrt_profile(str(output_dir).encode())
            print(f"profile: {n} file(s) written to {output_dir}")

    mod = types.ModuleType("antenv.axon_hooks")
    mod.get_axon_ntff_profile_hook = lambda: _hook
    mod.set_axon_ntff_profile_hook = lambda h: None
    sys.modules["antenv.axon_hooks"] = mod


def kernel(xyz, cholesky, opacity, features_dc):
    from concourse import bass_utils

    xyz = np.asarray(xyz, np.float32)
    cholesky = np.asarray(cholesky, np.float32)
    opacity = np.asarray(opacity, np.float32)
    features_dc = np.asarray(features_dc, np.float32)

    entries = _bin_entries(xyz, cholesky)
    groups = _pack_groups(entries)
    G = (len(groups) + N_CORES - 1) // N_CORES
    K = max(len(gr[1]) for gr in groups)

    # centered integer basis: exact in bf16
    gx = (np.arange(PIX) % TILE - 16).astype(np.float32)
    gy = (np.arange(PIX) // TILE - 16).astype(np.float32)
    basis = np.stack([gx * gx, gx * gy, gy * gy, gx, gy,
                      np.ones(PIX, np.float32)]).astype(ml_dtypes.bfloat16)
    ident = np.eye(SLOTS, dtype=np.float32)

    in_maps = []
    unpack = []  # per core: list of (g, j, t, ty, tx)
    for c in range(N_CORES):
        pm = np.zeros((SLOTS, G, 12), np.float32)
        mk = np.zeros((SLOTS, G, 3 * K), np.float32)
        um = []
        for g in range(G):
            gi = c + g * N_CORES
            if gi >= len(groups):
                continue
            for j, (ei, s0) in enumerate(groups[gi][1]):
                t, ty, tx, idxs = entries[ei]
                ns = len(idxs)
                ids = np.asarray(idxs)
                pm[s0:s0 + ns, g, 0:2] = xyz[t, ids]
                pm[s0:s0 + ns, g, 2:5] = cholesky[t, ids]
                pm[s0:s0 + ns, g, 5] = opacity[ids, 0]
                pm[s0:s0 + ns, g, 6:9] = features_dc[ids]
                pm[s0:s0 + ns, g, 9] = tx * TILE + 16.0
                pm[s0:s0 + ns, g, 10] = ty * TILE + 16.0
                mk[s0:s0 + ns, g, 3 * j:3 * j + 3] = 1.0
                um.append((g, j, t, ty, tx))
        in_maps.append({"params": pm.reshape(SLOTS, G * 12),
                        "basis": basis,
                        "msk": mk.reshape(SLOTS, G * 3 * K).astype(ml_dtypes.bfloat16),
                        "ident": ident})
        unpack.append(um)

    if (G, K) not in _CACHE:
        _CACHE[(G, K)] = _build_nc(G, K)
    nc = _CACHE[(G, K)]

    trace = bool(int(os.environ.get("GS_TRACE", "0")))
    if trace:
        _ensure_ntff_hook()
    res = bass_utils.run_bass_kernel_spmd(
        nc, in_maps, core_ids=list(range(N_CORES)), trace=trace)
    kernel.last_result = res

    img = np.zeros((T, 3, H, W), np.float32)
    for c in range(N_CORES):
        o = res.results[c]["out"]
        for (g, j, t, ty, tx) in unpack[c]:
            img[t, :, ty * TILE:(ty + 1) * TILE, tx * TILE:(tx + 1) * TILE] = \
                o[3 * j:3 * j + 3, g * PIX:(g + 1) * PIX].reshape(3, TILE, TILE)
    return img


# revision 16
# speedup vs baseline: 1.3556x; 1.0119x over previous
"""GaussianImage (Cholesky) renderer on 8 trn2 NeuronCores.

Tile-parallel over the pixel grid: the 256x256 image is cut into 32x32
tiles (64/frame, 128 total).  The host bins gaussians to tiles (pure
routing via a conservative support radius), then greedily bin-packs
whole tiles into 128-slot "groups" so the partition dim is ~full of real
gaussians (vs. the old one-tile-per-128-slots padding).  Each core runs
G groups.  All value math runs on device:

  per gaussian slot : sigmoid / conic / quadratic-basis coeffs   [Vector/Scalar]
  per group         : sigma = (c_hi + c_lo)(6,128)^T @ basis(6,1024)
                      -- two accumulating bf16 matmuls; the centered
                         integer basis is exact in bf16, and c_hi/c_lo
                         splitting restores fp32-level coefficients,
                         avoiding the 2-pass fp32 LOW_HIGH mode
                      alpha = Exp(-sigma)              -> bf16 [ScalarE]
                      img   = wblk(128,3K)^T @ alpha(128,1024)  [TensorE]
                      -- wblk is block-diagonal per tile (built from a
                         host-routed 0/1 mask so the SPMD program is
                         identical on every core)
                      out   = clamp(img, 0, 1)                  [VectorE]

Each pixel is owned by exactly one tile -> no cross-core reduction.
"""

import os
import numpy as np
import ml_dtypes

T, N, H, W = 2, 512, 256, 256
TILE = 32
NT = H // TILE          # 8 tiles per axis
N_CORES = 8
SLOTS = 128
PIX = TILE * TILE       # 1024
SIGMA_CUT = 18.0        # exp(-18) ~ 1.5e-8: invisible at fp32 image scale

_CACHE = {}


NWARM = 8               # PE warmup matmuls: ramp the p-state to 2.4 GHz


def _build_nc(G, K):
    import concourse.bass as bass
    import concourse.mybir as mybir
    from concourse.tile import TileContext
    import bass_rust

    f32 = mybir.dt.float32
    bf16 = mybir.dt.bfloat16
    Alu = mybir.AluOpType
    Act = mybir.ActivationFunctionType

    nc = bass.Bass("TRN2")
    params = nc.dram_tensor("params", [SLOTS, G * 12], f32, kind="ExternalInput")
    basis = nc.dram_tensor("basis", [6, PIX], bf16, kind="ExternalInput")
    msk = nc.dram_tensor("msk", [SLOTS, G * 3 * K], bf16, kind="ExternalInput")
    ident = nc.dram_tensor("ident", [SLOTS, SLOTS], f32, kind="ExternalInput")
    out = nc.dram_tensor("out", [3 * K, G * PIX], f32, kind="ExternalOutput")

    with TileContext(nc) as tc:
        with tc.tile_pool(name="const", bufs=1) as cpool, \
             tc.tile_pool(name="work", bufs=3) as wpool, \
             tc.tile_pool(name="ps_sig", bufs=4, space="PSUM") as ps_sig_pool, \
             tc.tile_pool(name="ps_img", bufs=3, space="PSUM") as ps_img_pool:

            p3 = cpool.tile([SLOTS, G, 12], f32, tag="params")
            bt = cpool.tile([6, PIX], bf16, tag="basis")
            mt = cpool.tile([SLOTS, G, 3 * K], bf16, tag="msk")
            it = cpool.tile([SLOTS, SLOTS], f32, tag="ident")
            # spread input DMAs over engine queues so they run concurrently
            nc.sync.dma_start(out=p3, in_=params[:].rearrange("p (g k) -> p g k", k=12))
            nc.gpsimd.dma_start(out=bt, in_=basis[:])
            nc.gpsimd.dma_start(out=mt, in_=msk[:].rearrange("p (g k) -> p g k", k=3 * K))
            nc.sync.dma_start(out=it, in_=ident[:])

            V = nc.vector
            S = nc.scalar
            GP = nc.gpsimd

            # warm the ACT table set while the params DMA is in flight
            warm = cpool.tile([SLOTS, 1], f32, tag="warm")
            S.memzero(warm)
            S.activation(warm, warm, Act.Sigmoid)

            def sc(tag):
                return cpool.tile([SLOTS, G], f32, tag=tag, name=tag)

            cth = cpool.tile([SLOTS, G, 6], f32, tag="coef")
            wsg = cpool.tile([SLOTS, G, 3], f32, tag="wsg")
            w3 = cpool.tile([SLOTS, G, 3], f32, tag="w3")

            # centers: cx = 0.5*W*(tanh(z)+1) = W*sigmoid(2z); offsets absorbed
            sx, sy = sc("sx"), sc("sy")
            S.activation(sx, p3[:, :, 0], Act.Sigmoid, scale=2.0)
            S.activation(sy, p3[:, :, 1], Act.Sigmoid, scale=2.0)
            # sxy^2 and syy parts via ScalarE square (bias folds the +0.5)
            hf = cpool.tile([SLOTS, 1], f32, tag="hf")
            GP.memset(hf, 0.5)
            t2, t3 = sc("t2"), sc("t3")
            S.activation(t2, p3[:, :, 3], Act.Square)
            S.activation(t3, p3[:, :, 4], Act.Square, bias=hf)
            osg = sc("osg")
            S.activation(osg, p3[:, :, 5], Act.Sigmoid)
            S.activation(wsg, p3[:, :, 6:9], Act.Sigmoid)
            # pre-load the Exp ACT table while ScalarE is otherwise idle so it
            # doesn't serialize inside the hot loop.  Reading wsg (the last
            # sigmoid output) pins this AFTER all sigmoid-set activations —
            # ScalarE holds one table set, every set switch costs ~1.3us.
            S.activation(warm, wsg[:, 0, 0:1], Act.Exp)

            # conic coefficient chain: det path + linear terms on Vector,
            # head + quadratic branch on GpSimd (plain tensor-tensor only)
            a0 = sc("a0")
            GP.tensor_add(out=a0, in0=p3[:, :, 2], in1=hf.broadcast_to([SLOTS, G]))
            a1 = p3[:, :, 3]
            t0, t1, v = sc("t0"), sc("t1"), sc("v")
            GP.tensor_mul(out=t0, in0=a0, in1=a0)
            GP.tensor_mul(out=t1, in0=a0, in1=a1)
            GP.tensor_mul(out=v, in0=t1, in1=t1)
            ex, ey = sc("ex"), sc("ey")
            V.scalar_tensor_tensor(out=ex, in0=sx, scalar=float(W), in1=p3[:, :, 9],
                                   op0=Alu.mult, op1=Alu.subtract)
            V.scalar_tensor_tensor(out=ey, in0=sy, scalar=float(H), in1=p3[:, :, 10],
                                   op0=Alu.mult, op1=Alu.subtract)
            syy = sc("syy")
            V.tensor_add(out=syy, in0=t2, in1=t3)
            u, det, rdet = sc("u"), sc("det"), sc("rdet")
            V.tensor_mul(out=u, in0=t0, in1=syy)
            V.tensor_sub(out=det, in0=u, in1=v)
            V.reciprocal(out=rdet, in_=det)
            # cth0 = 0.5*ca, cth1 = cb = -sxy/det, cth2 = 0.5*cc
            V.scalar_tensor_tensor(out=cth[:, :, 0], in0=syy, scalar=0.5, in1=rdet,
                                   op0=Alu.mult, op1=Alu.mult)
            V.scalar_tensor_tensor(out=cth[:, :, 1], in0=t1, scalar=-1.0, in1=rdet,
                                   op0=Alu.mult, op1=Alu.mult)
            V.scalar_tensor_tensor(out=cth[:, :, 2], in0=t0, scalar=0.5, in1=rdet,
                                   op0=Alu.mult, op1=Alu.mult)
            # linear terms on Vector via fused scalar-tensor-tensor:
            # coef_x = (-2ex)*cth0 + (-ey)*cth1, coef_y = (-2ey)*cth2 + (-ex)*cth1
            m1, m2 = sc("m1"), sc("m2")
            V.scalar_tensor_tensor(out=m1, in0=ex, scalar=-2.0, in1=cth[:, :, 0],
                                   op0=Alu.mult, op1=Alu.mult)
            V.scalar_tensor_tensor(out=m2, in0=ey, scalar=-1.0, in1=cth[:, :, 1],
                                   op0=Alu.mult, op1=Alu.mult)
            V.tensor_add(out=cth[:, :, 3], in0=m1, in1=m2)
            m3, m4 = sc("m3"), sc("m4")
            V.scalar_tensor_tensor(out=m3, in0=ey, scalar=-2.0, in1=cth[:, :, 2],
                                   op0=Alu.mult, op1=Alu.mult)
            V.scalar_tensor_tensor(out=m4, in0=ex, scalar=-1.0, in1=cth[:, :, 1],
                                   op0=Alu.mult, op1=Alu.mult)
            V.tensor_add(out=cth[:, :, 4], in0=m3, in1=m4)
            # quadratic branch on GpSimd: cth5 = cth0*ex^2 + cth1*ex*ey + cth2*ey^2
            exx, exy, eyy = sc("exx"), sc("exy"), sc("eyy")
            GP.tensor_mul(out=exx, in0=ex, in1=ex)
            GP.tensor_mul(out=exy, in0=ex, in1=ey)
            GP.tensor_mul(out=eyy, in0=ey, in1=ey)
            p1, p2, p3b, q = sc("p1"), sc("p2"), sc("p3b"), sc("q")
            GP.tensor_mul(out=p1, in0=cth[:, :, 0], in1=exx)
            GP.tensor_mul(out=p2, in0=cth[:, :, 1], in1=exy)
            GP.tensor_add(out=q, in0=p1, in1=p2)
            GP.tensor_mul(out=p3b, in0=cth[:, :, 2], in1=eyy)
            GP.tensor_add(out=cth[:, :, 5], in0=q, in1=p3b)

            # colors*opacity, scattered block-diagonally through the mask
            # (0-stride broadcasts turn 27 tiny muls into 1 + G)
            V.tensor_mul(out=w3, in0=wsg,
                         in1=osg.unsqueeze(2).broadcast_to([SLOTS, G, 3]))
            wblk = cpool.tile([SLOTS, G, 3 * K], bf16, tag="wblk")
            for g in range(G):
                GP.tensor_mul(
                    out=wblk[:, g, :].rearrange("p (k c) -> p k c", c=3),
                    in0=w3[:, g, :].unsqueeze(1).broadcast_to([SLOTS, K, 3]),
                    in1=mt[:, g, :].rearrange("p (k c) -> p k c", c=3))

            # per-group transpose of coeffs -> [6,128] at partition 0 (no
            # shuffle DMA needed; transposed tiles are valid lhsT operands)
            chis, clos = [], []
            for g in range(G):
                tpg = ps_img_pool.tile([6, SLOTS], f32, tag="img", name=f"tp{g}")
                nc.tensor.transpose(tpg, cth[:, g, :], it)
                chi = cpool.tile([6, SLOTS], bf16, tag=f"chi{g}", name=f"chi{g}")
                clo = cpool.tile([6, SLOTS], bf16, tag=f"clo{g}", name=f"clo{g}")
                S.copy(out=chi, in_=tpg)
                V.tensor_sub(out=clo, in0=tpg, in1=chi)
                chis.append(chi)
                clos.append(clo)

            st = cpool.tile([3 * K, G * PIX], f32, tag="stage")
            alphas = []

            # --- hot loop, PE-dense order: sig0 sig1 img0 sig2 img1 img2;
            # exp and clamp pipelined at 512-column half granularity ---
            def do_sigma(g):
                alpha = wpool.tile([SLOTS, PIX], bf16, tag="alpha", name=f"alpha{g}")
                for h in range(2):
                    cs = slice(512 * h, 512 * (h + 1))
                    sig = ps_sig_pool.tile([SLOTS, 512], f32, tag="sig", name=f"sig{g}h{h}")
                    nc.tensor.matmul(sig, chis[g], bt[:, cs], start=True, stop=False)
                    nc.tensor.matmul(sig, clos[g], bt[:, cs], start=False, stop=True)
                    S.activation(alpha[:, cs], sig, Act.Exp, scale=-1.0)
                alphas.append(alpha)

            def do_img(g):
                wre = wblk[:, g, :]
                o0 = g * PIX
                for h in range(2):
                    cs = slice(512 * h, 512 * (h + 1))
                    img = ps_img_pool.tile([3 * K, 512], f32, tag="img", name=f"img{g}h{h}")
                    nc.tensor.matmul(img, wre, alphas[g][:, cs], start=True, stop=True)
                    V.tensor_scalar(out=st[:, o0 + 512 * h:o0 + 512 * (h + 1)], in0=img,
                                    scalar1=0.0, scalar2=1.0, op0=Alu.max, op1=Alu.min)
                    nc.sync.dma_start(out=out[:, o0 + 512 * h:o0 + 512 * (h + 1)],
                                      in_=st[:, o0 + 512 * h:o0 + 512 * (h + 1)])

            # sigmas first, then imgs: the tail group's exp/img/clamp overlap
            # the earlier groups' img matmuls instead of serializing at the end
            for g in range(G):
                do_sigma(g)
            for g in range(G):
                do_img(g)

    bass_rust.generate_event_semaphores(nc)
    return nc


def _bin_entries(xyz, cholesky):
    """Host-side routing: which gaussians overlap which 32x32 tile."""
    means = np.tanh(xyz.astype(np.float64))
    cx = 0.5 * W * (means[..., 0] + 1.0)
    cy = 0.5 * H * (means[..., 1] + 1.0)
    chol = cholesky.astype(np.float64) + np.array([0.5, 0.0, 0.5])
    l0, l1, l2 = chol[..., 0], chol[..., 1], chol[..., 2]
    sxx, sxy, syy = l0 * l0, l0 * l1, l1 * l1 + l2 * l2
    tr, det = sxx + syy, sxx * syy - sxy * sxy
    lam = tr / 2 + np.sqrt(np.maximum(tr * tr / 4 - det, 0.0))
    r = np.sqrt(2.0 * SIGMA_CUT * np.maximum(lam, 0.0)) + 1.0

    entries = []  # (frame, ty, tx, index-list)
    for t in range(T):
        x0 = np.clip(((cx[t] - r[t]) // TILE).astype(int), 0, NT - 1)
        x1 = np.clip(((cx[t] + r[t]) // TILE).astype(int), 0, NT - 1)
        y0 = np.clip(((cy[t] - r[t]) // TILE).astype(int), 0, NT - 1)
        y1 = np.clip(((cy[t] + r[t]) // TILE).astype(int), 0, NT - 1)
        buckets = [[[] for _ in range(NT)] for _ in range(NT)]
        for n in range(N):
            for ty in range(y0[n], y1[n] + 1):
                for tx in range(x0[n], x1[n] + 1):
                    buckets[ty][tx].append(n)
        for ty in range(NT):
            for tx in range(NT):
                assert len(buckets[ty][tx]) <= SLOTS, "tile overflow: >128 gaussians"
                if buckets[ty][tx]:
                    entries.append((t, ty, tx, buckets[ty][tx]))
    return entries


def _pack_groups(entries):
    """Greedy first-fit-decreasing bin pack of tiles into 128-slot groups."""
    order = sorted(range(len(entries)), key=lambda i: -len(entries[i][3]))
    groups = []  # [used_slots, [(entry_idx, slot_start), ...]]
    for i in order:
        c = len(entries[i][3])
        for gr in groups:
            if gr[0] + c <= SLOTS:
                gr[1].append((i, gr[0]))
                gr[0] += c
                break
        else:
            groups.append([c, [(i, 0)]])
    return groups


def _ensure_ntff_hook():
    """Provide antenv.axon_hooks (missing in this image) so trace=True works."""
    import sys, types, ctypes, contextlib
    if "antenv.axon_hooks" in sys.modules:
        return
    so_path = "/opt/axon/libaxon_pjrt.so"
    if not os.path.exists(so_path):
        return
    lib = ctypes.CDLL(so_path)
    if not hasattr(lib, "axon_start_nrt_profile"):
        return
    lib.axon_start_nrt_profile.argtypes = [ctypes.POINTER(ctypes.c_int64), ctypes.c_size_t]
    lib.axon_start_nrt_profile.restype = ctypes.c_int64
    lib.axon_stop_nrt_profile.argtypes = [ctypes.c_char_p]
    lib.axon_stop_nrt_profile.restype = ctypes.c_int64

    @contextlib.contextmanager
    def _hook(output_dir, device_ids):
        import jax
        jax.devices()
        if device_ids:
            ids = (ctypes.c_int64 * len(device_ids))(*device_ids)
            rc = lib.axon_start_nrt_profile(ids, len(device_ids))
        else:
            rc = lib.axon_start_nrt_profile(None, 0)
        if rc != 0:
            raise RuntimeError(f"axon_start_nrt_profile rc={rc}")
        try:
            yield
        finally:
            n = lib.axon_stop_nrt_profile(str(output_dir).encode())
            print(f"profile: {n} file(s) written to {output_dir}")

    mod = types.ModuleType("antenv.axon_hooks")
    mod.get_axon_ntff_profile_hook = lambda: _hook
    mod.set_axon_ntff_profile_hook = lambda h: None
    sys.modules["antenv.axon_hooks"] = mod


def kernel(xyz, cholesky, opacity, features_dc):
    from concourse import bass_utils

    xyz = np.asarray(xyz, np.float32)
    cholesky = np.asarray(cholesky, np.float32)
    opacity = np.asarray(opacity, np.float32)
    features_dc = np.asarray(features_dc, np.float32)

    entries = _bin_entries(xyz, cholesky)
    groups = _pack_groups(entries)
    G = (len(groups) + N_CORES - 1) // N_CORES
    K = max(len(gr[1]) for gr in groups)

    # centered integer basis: exact in bf16
    gx = (np.arange(PIX) % TILE - 16).astype(np.float32)
    gy = (np.arange(PIX) // TILE - 16).astype(np.float32)
    basis = np.stack([gx * gx, gx * gy, gy * gy, gx, gy,
                      np.ones(PIX, np.float32)]).astype(ml_dtypes.bfloat16)
    ident = np.eye(SLOTS, dtype=np.float32)

    in_maps = []
    unpack = []  # per core: list of (g, j, t, ty, tx)
    for c in range(N_CORES):
        pm = np.zeros((SLOTS, G, 12), np.float32)
        mk = np.zeros((SLOTS, G, 3 * K), np.float32)
        um = []
        for g in range(G):
            gi = c + g * N_CORES
            if gi >= len(groups):
                continue
            for j, (ei, s0) in enumerate(groups[gi][1]):
                t, ty, tx, idxs = entries[ei]
                ns = len(idxs)
                ids = np.asarray(idxs)
                pm[s0:s0 + ns, g, 0:2] = xyz[t, ids]
                pm[s0:s0 + ns, g, 2:5] = cholesky[t, ids]
                pm[s0:s0 + ns, g, 5] = opacity[ids, 0]
                pm[s0:s0 + ns, g, 6:9] = features_dc[ids]
                pm[s0:s0 + ns, g, 9] = tx * TILE + 16.0
                pm[s0:s0 + ns, g, 10] = ty * TILE + 16.0
                mk[s0:s0 + ns, g, 3 * j:3 * j + 3] = 1.0
                um.append((g, j, t, ty, tx))
        in_maps.append({"params": pm.reshape(SLOTS, G * 12),
                        "basis": basis,
                        "msk": mk.reshape(SLOTS, G * 3 * K).astype(ml_dtypes.bfloat16),
                        "ident": ident})
        unpack.append(um)

    if (G, K) not in _CACHE:
        _CACHE[(G, K)] = _build_nc(G, K)
    nc = _CACHE[(G, K)]

    trace = bool(int(os.environ.get("GS_TRACE", "0")))
    if trace:
        _ensure_ntff_hook()
    res = bass_utils.run_bass_kernel_spmd(
        nc, in_maps, core_ids=list(range(N_CORES)), trace=trace)
    kernel.last_result = res

    img = np.zeros((T, 3, H, W), np.float32)
    for c in range(N_CORES):
        o = res.results[c]["out"]
        for (g, j, t, ty, tx) in unpack[c]:
            img[t, :, ty * TILE:(ty + 1) * TILE, tx * TILE:(tx + 1) * TILE] = \
                o[3 * j:3 * j + 3, g * PIX:(g + 1) * PIX].reshape(3, TILE, TILE)
    return img
